# revision 5
# baseline (speedup 1.0000x reference)
"""Trainium2 Bass kernel for nn_MultiHeadAttention_52991306498287.

Computes three cross-attentions (mm, md, dd) over medications/diagnoses
with both additive and post-softmax multiplicative masks, returning
outputs AND full attention-weight matrices.

Sharding: data-parallel over batch. B=16 -> 2 batches per core x 8 cores.
All weights replicated.

Self-contained: hardcodes shapes; imports bass from /opt/trn_rl_repo
(also importable via the preset PYTHONPATH .axon_site mirror).
"""
import sys

for _p in ("/opt/trn_rl_repo", "/root/.axon_site/_ro/trn_rl_repo"):
    if _p not in sys.path:
        sys.path.append(_p)

import numpy as np
import concourse.bass as bass
import concourse.tile as tile
from concourse import bacc, mybir
from concourse.bass_utils import run_bass_kernel_spmd
from concourse.masks import make_identity

F32 = mybir.dt.float32
F32R = mybir.dt.float32r
BF16 = mybir.dt.bfloat16
AF = mybir.ActivationFunctionType
ALU = mybir.AluOpType

B, S, D, H = 16, 1024, 16, 4
DH = D // H                  # 4
N_CORES = 8
BL = B // N_CORES            # 2 batches per core
NCH = S // 128               # 8 chunks of 128

PROJ = ["mq", "mk", "mv", "dq", "dk", "dv", "o"]
# variant -> (q source tensor, k source, v source); m=0, d=1
VARIANTS = [("mm", 0, 0, 0), ("md", 0, 1, 1), ("dd", 1, 1, 1)]

_CACHED_NC = None


def build_nc():
    nc = bacc.Bacc("TRN2", target_bir_lowering=False, debug=False,
                   enable_asserts=True, num_devices=N_CORES)

    med_d = nc.dram_tensor("medications", [BL, S, D], F32, kind="ExternalInput").ap()
    diag_d = nc.dram_tensor("diagnoses", [BL, S, D], F32, kind="ExternalInput").ap()
    mask_d = nc.dram_tensor("mask", [BL, 1, S, S], F32, kind="ExternalInput").ap()
    w_in = {}
    b_in = {}
    for p in PROJ:
        w_in[p] = nc.dram_tensor("w" + p, [D, D], F32, kind="ExternalInput").ap()
        b_in[p] = nc.dram_tensor("b" + p, [D], F32, kind="ExternalInput").ap()

    out_d = {}
    wgt_d = {}
    for vname, _, _, _ in VARIANTS:
        out_d[vname] = nc.dram_tensor(
            "out_" + vname, [BL, S, D], F32, kind="ExternalOutput").ap()
        wgt_d[vname] = nc.dram_tensor(
            "w_" + vname, [BL, H, S, S], F32, kind="ExternalOutput").ap()

    x_d = [med_d, diag_d]

    with tile.TileContext(nc) as tc:
        with (
            tc.tile_pool(name="const", bufs=1) as constp,
            tc.tile_pool(name="mnp", bufs=1) as mnp,
            tc.tile_pool(name="abp", bufs=1) as abp,
            tc.tile_pool(name="tmpp", bufs=2) as tmpp,
            tc.tile_pool(name="ep", bufs=3) as ep,
            tc.tile_pool(name="eap", bufs=3) as eap,
            tc.tile_pool(name="wp", bufs=4) as wp,
            tc.tile_pool(name="qkp", bufs=2) as qkp,
            tc.tile_pool(name="sm", bufs=2) as sm,
            tc.tile_pool(name="psl", bufs=2, space="PSUM") as psl,
            tc.tile_pool(name="psw", bufs=2, space="PSUM") as psw,
            tc.tile_pool(name="pso", bufs=1, space="PSUM") as pso,
        ):
            # ---------------- constants ----------------
            id_f32 = constp.tile([128, 128], F32, tag="idf")
            make_identity(nc, id_f32[:])
            id_bf16 = constp.tile([128, 128], BF16, tag="idb")
            make_identity(nc, id_bf16[:])
            ones5 = constp.tile([128, DH + 1], BF16, tag="onesc")
            nc.gpsimd.memset(ones5[:], 0.0)
            nc.gpsimd.memset(ones5[:, DH:DH + 1], 1.0)

            # augmented weights [17, 16]: rows 0:16 = W, row 16 = bias
            waug = {}
            waug_pad = {}
            for p in PROJ:
                wf = constp.tile([D + 1, D], F32, tag="wf_" + p)
                nc.sync.dma_start(wf[0:D, :], w_in[p][:])
                nc.sync.dma_start(wf[D:D + 1, :], b_in[p][None, :])
                if p in ("mq", "dq"):   # fold 1/sqrt(depth) into Q projection
                    nc.vector.tensor_scalar(out=wf[:], in0=wf[:], scalar1=0.5,
                                            scalar2=None, op0=ALU.mult)
                wr = constp.tile([D + 1, D], F32R, tag="wr_" + p)
                nc.vector.tensor_copy(wr[:], wf[:])
                waug[p] = wr
                if p in ("mq", "dq", "mk", "dk"):
                    pads = []
                    for pair in range(H // 2):
                        wpadf = constp.tile([D + 1, 64], F32,
                                            tag=f"wpf_{p}_{pair}")
                        nc.gpsimd.memset(wpadf[:], 0.0)
                        for j in range(2):
                            h = 2 * pair + j
                            nc.vector.tensor_copy(
                                wpadf[:, 32 * j:32 * j + DH],
                                wf[:, DH * h:DH * (h + 1)])
                        wpadr = constp.tile([D + 1, 64], F32R,
                                            tag=f"wpr_{p}_{pair}")
                        nc.vector.tensor_copy(wpadr[:], wpadf[:])
                        pads.append(wpadr)
                    waug_pad[p] = pads

            for b in range(BL):
                # ---------------- masks: A = 1-maskT, Bn = 1-mask (bf16) ------
                mn = mnp.tile([128, NCH, S], F32, tag="mn")
                for kc in range(NCH):
                    nc.sync.dma_start(
                        mn[:, kc, :], mask_d[b, 0, kc * 128:(kc + 1) * 128, :])
                a_t = abp.tile([128, NCH, S], BF16, tag="a")
                b_t = abp.tile([128, NCH, S], BF16, tag="b")
                for kc in range(NCH):
                    zt_ps = psl.tile([128, S], F32, tag="psl")
                    for qc in range(NCH):
                        nc.tensor.transpose(
                            zt_ps[:, qc * 128:(qc + 1) * 128],
                            mn[:, qc, kc * 128:(kc + 1) * 128], id_f32[:])
                    nc.vector.tensor_scalar(
                        out=a_t[:, kc, :], in0=zt_ps[:], scalar1=-1.0,
                        scalar2=1.0, op0=ALU.mult, op1=ALU.add)
                    nc.vector.tensor_scalar(
                        out=b_t[:, kc, :], in0=mn[:, kc, :], scalar1=-1.0,
                        scalar2=1.0, op0=ALU.mult, op1=ALU.add)

                # ---------------- projections (per source tensor) -------------
                qk32 = {}     # (src, 'q'|'k') -> [128, S] bf16, head h at partition 32h
                v_sb = {}     # src -> [128, NCH, 16] bf16
                for t in range(2):   # 0=med, 1=diag
                    x_sb = sm.tile([128, NCH, D + 1], F32, tag="xsb")
                    nc.sync.dma_start(
                        x_sb[:, :, 0:D],
                        x_d[t][b].rearrange("(t p) c -> p t c", p=128))
                    nc.gpsimd.memset(x_sb[:, :, D:D + 1], 1.0)
                    xta = sm.tile([D + 1, S], F32R, tag="xta")
                    for half in range(2):
                        xt_ps = psw.tile([D + 1, 512], F32, tag="psw")
                        for j in range(4):
                            st = half * 4 + j
                            nc.tensor.transpose(
                                xt_ps[:, j * 128:(j + 1) * 128],
                                x_sb[:, st, :], id_f32[:])
                        nc.vector.tensor_copy(
                            xta[:, half * 512:(half + 1) * 512], xt_ps[:])

                    pq = "mq" if t == 0 else "dq"
                    pk = "mk" if t == 0 else "dk"
                    pv = "mv" if t == 0 else "dv"
                    for role, p in (("q", pq), ("k", pk)):
                        pair_tiles = []
                        for pair in range(H // 2):
                            ps_qk = psl.tile([64, S], F32, tag="psl")
                            for half in range(2):
                                nc.tensor.matmul(
                                    ps_qk[:, half * 512:(half + 1) * 512],
                                    lhsT=waug_pad[p][pair][:],
                                    rhs=xta[:, half * 512:(half + 1) * 512])
                            qk = qkp.tile([64, S], BF16,
                                          tag=f"qk_{t}_{role}_{pair}")
                            nc.vector.tensor_copy(qk[:], ps_qk[:])
                            pair_tiles.append(qk)
                        qk32[(t, role)] = pair_tiles

                    ps_v = psw.tile([128, 128], F32, tag="psw")
                    for st in range(NCH):
                        nc.tensor.matmul(
                            ps_v[:, st * D:(st + 1) * D],
                            lhsT=xta[:, st * 128:(st + 1) * 128],
                            rhs=waug[pv][:])
                    vv = sm.tile([128, NCH, D], BF16, tag=f"v_{t}")
                    nc.vector.tensor_copy(
                        vv[:], ps_v[:].rearrange("p (t c) -> p t c", c=D))
                    v_sb[t] = vv

                # ---------------- attention ----------------
                for vname, qs, ks, vs in VARIANTS:
                    att = sm.tile([128, NCH, D + 1], F32, tag="att")
                    nc.gpsimd.memset(att[:, :, D:D + 1], 1.0)
                    for h in range(H):
                        tmp = tmpp.tile([128, NCH, S], BF16, tag="tmp")
                        o5_ps = pso.tile([DH + 1, S], F32, tag="o5")
                        ea_keep = []
                        for kc in range(NCH):
                            l_ps = psl.tile([128, S], F32, tag="psl")
                            pr, jo = h // 2, 32 * (h % 2)
                            for half in range(2):
                                nc.tensor.matmul(
                                    l_ps[:, half * 512:(half + 1) * 512],
                                    lhsT=qk32[(ks, "k")][pr][
                                        jo:jo + DH,
                                        kc * 128:(kc + 1) * 128],
                                    rhs=qk32[(qs, "q")][pr][
                                        jo:jo + DH,
                                        half * 512:(half + 1) * 512])
                            e = ep.tile([128, S], BF16, tag="e")
                            nc.scalar.activation(e[:], l_ps[:], AF.Exp)
                            ea = eap.tile([128, S], BF16, tag="ea")
                            nc.vector.tensor_tensor(
                                out=ea[:], in0=e[:], in1=a_t[:, kc, :],
                                op=ALU.mult)
                            nc.vector.tensor_tensor(
                                out=tmp[:, kc, :], in0=ea[:],
                                in1=b_t[:, kc, :], op=ALU.mult)
                            for half in range(2):
                                h0, h1 = half * 512, (half + 1) * 512
                                nc.tensor.matmul(
                                    o5_ps[0:DH + 1, h0:h1],
                                    lhsT=ones5[:],
                                    rhs=ea[:, h0:h1],
                                    start=(kc == 0), stop=(kc == NCH - 1),
                                    skip_group_check=True)
                                nc.tensor.matmul(
                                    o5_ps[0:DH, h0:h1],
                                    lhsT=v_sb[vs][:, kc, DH * h:DH * (h + 1)],
                                    rhs=tmp[:, kc, h0:h1],
                                    start=(kc == 0), stop=(kc == NCH - 1),
                                    skip_group_check=True)
                            ea_keep.append(ea)

                        o5_sb = sm.tile([DH + 1, S], F32, tag="o5sb")
                        nc.vector.tensor_copy(o5_sb[:], o5_ps[:])
                        t5_ps = psw.tile([128, NCH * (DH + 1)], F32, tag="psw")
                        for qt in range(NCH):
                            nc.tensor.transpose(
                                t5_ps[:, qt * (DH + 1):(qt + 1) * (DH + 1)],
                                o5_sb[:, qt * 128:(qt + 1) * 128],
                                id_f32[0:DH + 1, 0:DH + 1])
                        recip = sm.tile([128, NCH], F32, tag="recip")
                        t5v = t5_ps[:].rearrange("p (q f) -> p q f", f=DH + 1)
                        nc.vector.reciprocal(recip[:], t5v[:, :, DH])
                        for qt in range(NCH):
                            nc.vector.tensor_scalar(
                                out=att[:, qt, DH * h:DH * (h + 1)],
                                in0=t5v[:, qt, 0:DH],
                                scalar1=recip[:, qt:qt + 1], scalar2=None,
                                op0=ALU.mult)

                        for qt in range(NCH):
                            w_ps = psw.tile([128, S], BF16, tag="psw")
                            for kc in range(NCH):
                                nc.tensor.transpose(
                                    w_ps[:, kc * 128:(kc + 1) * 128],
                                    tmp[:, kc, qt * 128:(qt + 1) * 128],
                                    id_bf16[:])
                            w_sb = wp.tile([128, S], F32, tag="w")
                            if qt % 2 == 0:
                                nc.scalar.activation(
                                    w_sb[:], w_ps[:], AF.Copy, bias=0.0,
                                    scale=recip[:, qt:qt + 1])
                            else:
                                nc.vector.tensor_scalar(
                                    out=w_sb[:], in0=w_ps[:],
                                    scalar1=recip[:, qt:qt + 1],
                                    scalar2=None, op0=ALU.mult)
                            nc.sync.dma_start(
                                wgt_d[vname][b, h, qt * 128:(qt + 1) * 128, :],
                                w_sb[:])

                    # ---- output projection for this (b, variant) ----
                    atta = sm.tile([D + 1, S], F32R, tag="atta")
                    for half in range(2):
                        at_ps = psw.tile([D + 1, 512], F32, tag="psw")
                        for j in range(4):
                            qt = half * 4 + j
                            nc.tensor.transpose(
                                at_ps[:, j * 128:(j + 1) * 128],
                                att[:, qt, :], id_f32[:])
                        nc.vector.tensor_copy(
                            atta[:, half * 512:(half + 1) * 512], at_ps[:])
                    pj_ps = psw.tile([128, 128], F32, tag="psw")
                    for qt in range(NCH):
                        nc.tensor.matmul(
                            pj_ps[:, qt * D:(qt + 1) * D],
                            lhsT=atta[:, qt * 128:(qt + 1) * 128],
                            rhs=waug["o"][:])
                    ob = sm.tile([128, NCH, D], F32, tag="ob")
                    nc.vector.tensor_copy(
                        ob[:], pj_ps[:].rearrange("p (t c) -> p t c", c=D))
                    nc.sync.dma_start(
                        out_d[vname][b].rearrange("(t p) c -> p t c", p=128),
                        ob[:])
    nc.compile()
    return nc


def _get_nc():
    global _CACHED_NC
    if _CACHED_NC is None:
        _CACHED_NC = build_nc()
    return _CACHED_NC


def kernel(medications, diagnoses, mask,
           wmq, bmq, wmk, bmk, wmv, bmv,
           wdq, bdq, wdk, bdk, wdv, bdv,
           wo, bo, _trace=False):
    nc = _get_nc()
    med = np.asarray(medications, np.float32)
    diag = np.asarray(diagnoses, np.float32)
    msk = np.asarray(mask, np.float32)
    wts = dict(
        wmq=np.asarray(wmq, np.float32), bmq=np.asarray(bmq, np.float32),
        wmk=np.asarray(wmk, np.float32), bmk=np.asarray(bmk, np.float32),
        wmv=np.asarray(wmv, np.float32), bmv=np.asarray(bmv, np.float32),
        wdq=np.asarray(wdq, np.float32), bdq=np.asarray(bdq, np.float32),
        wdk=np.asarray(wdk, np.float32), bdk=np.asarray(bdk, np.float32),
        wdv=np.asarray(wdv, np.float32), bdv=np.asarray(bdv, np.float32),
        wo=np.asarray(wo, np.float32), bo=np.asarray(bo, np.float32),
    )
    in_maps = []
    for i in range(N_CORES):
        s = slice(BL * i, BL * (i + 1))
        m = dict(medications=med[s], diagnoses=diag[s], mask=msk[s])
        m.update(wts)
        in_maps.append(m)

    res = run_bass_kernel_spmd(nc, in_maps, list(range(N_CORES)),
                               trace=_trace)
    outs = []
    for name in ("out_mm", "w_mm", "out_md", "w_md", "out_dd", "w_dd"):
        outs.append(np.concatenate(
            [res.results[i][name] for i in range(N_CORES)], axis=0))
    if _trace:
        return tuple(outs), res
    return tuple(outs)
